# revision 16
# baseline (speedup 1.0000x reference)
"""Trainium2 Bass kernel for the transit-network cost module.

Contract: kernel(**inputs) takes FULL unsharded numpy inputs (B=64) and
returns the FULL output tuple matching reference.reference(...).

Strategy: pure data parallel over 8 NeuronCores (8 batch elements each).
Per core, each batch element's 600x600 = 360000-element plane is viewed as
16 partitions x 22500 columns, so 8 batch elements exactly fill the 128
SBUF partitions.  The heavy elementwise+reduction work is done with fused
DVE scalar_tensor_tensor ops (elementwise out + per-partition accum in one
pass), bf16 for the reduction-only streams, fp32 for the exact trip_times
output.  Per-batch scalars are produced by folding the 128 per-partition
partial sums with a one-hot [128,8] matmul on the tensor engine.
"""

import os

import numpy as np

B = 64
N = 600
R = 40
L = 50
MIN_ROUTE_LEN, MAX_ROUTE_LEN = 2, 48

N_CORES = 8
B_LOC = B // N_CORES            # 8 batch elements per core
PGRP = 16                       # partitions per batch element
COLS = (N * N) // PGRP          # 22500 columns per partition-stream
CHUNK = 2500                    # columns per chunk
NCHUNK = COLS // CHUNK          # 9
MM = 500                        # matmul moving free-dim sub-chunk
NTERM = 6                       # accumulated STT terms

_CACHE = {}
LAST_RESULTS = None


def _split_multi_waits(nc, max_waits=1):
    """walrus in this toolchain encodes at most one sync-wait per engine
    instruction; Tile emits several.  Move extra waits onto standalone
    EventSemaphore instructions inserted immediately before the consumer
    (same engine, same position => identical synchronization semantics)."""
    import concourse.mybir as mybir
    import bass_rust
    # spare sem for the carriers' mandatory update; nothing waits on it
    used = set()
    for f in nc.m.functions:
        for bb in f.blocks:
            for ins in bb.instructions:
                si = ins.sync_info
                if si is not None:
                    for w in si.on_wait:
                        used.add(w.id)
                    for u in si.on_update:
                        used.add(u.id)
    spare = max([i for i in used if i is not None], default=150) + 1
    assert spare <= 255, spare
    n = 0
    for f in nc.m.functions:
        for bb in f.blocks:
            out = []
            changed = False
            for ins in bb.instructions:
                si = ins.sync_info
                if si is not None and len(si.on_wait) > max_waits:
                    waits = list(si.on_wait)
                    extra, keep = waits[:-max_waits], waits[-max_waits:]
                    for j, w in enumerate(extra):
                        ev = mybir.InstDrain(
                            name=f"{ins.name}-w{j}", ins=[], outs=[])
                        ev.engine = ins.engine
                        ev.sync_info = bass_rust.SyncInfo(
                            on_wait=[w],
                            on_update=[bass_rust.SyncUpdate(
                                sync_type="semaphore", id=spare,
                                ant_name=f"splitw_{spare}",
                                update_mode="sem-inc", update_value=1,
                                update_reg=None)])
                        out.append(ev)
                        n += 1
                    ins.sync_info = bass_rust.SyncInfo(
                        on_wait=keep, on_update=list(si.on_update))
                    changed = True
                out.append(ins)
            if changed:
                bb.instructions = out
    return n


def _build_nc():
    import concourse.bass as bass
    import concourse.mybir as mybir
    from concourse.tile import TileContext

    dt = mybir.dt
    Alu = mybir.AluOpType
    f32, bf16, i32, u8 = dt.float32, dt.bfloat16, dt.int32, dt.uint8

    nc = bass.Bass(trn_type="TRN2", enable_partition_id=False)

    # ---- I/O ----
    dm = nc.dram_tensor("demand", [B_LOC, N, N], f32, kind="ExternalInput")
    tt = nc.dram_tensor("transit_times", [B_LOC, N, N], f32, kind="ExternalInput")
    ntr = nc.dram_tensor("n_transfers", [B_LOC, N, N], i32, kind="ExternalInput")
    hp = nc.dram_tensor("has_path", [B_LOC, N, N], u8, kind="ExternalInput")
    br = nc.dram_tensor("batch_routes", [B_LOC, R, L], i32, kind="ExternalInput")
    nrl = nc.dram_tensor("n_routes_left_to_plan", [B_LOC, 1], i32, kind="ExternalInput")
    hcr = nc.dram_tensor("has_current_route", [B_LOC, 1], u8, kind="ExternalInput")
    # one-hot fold constants (host-precomputed)
    doh_bf = nc.dram_tensor("oh16_bf16", [128, B_LOC], bf16, kind="ExternalInput")
    doh_f = nc.dram_tensor("oh16_f32", [128, B_LOC], f32, kind="ExternalInput")
    dohr = nc.dram_tensor("ohr40", [128, 3 * B_LOC], f32, kind="ExternalInput")

    o_tt = nc.dram_tensor("trip_times", [B_LOC, N, N], f32, kind="ExternalOutput")
    o_tdt = nc.dram_tensor("total_demand_time", [B_LOC, 1], f32, kind="ExternalOutput")
    o_tat = nc.dram_tensor("trips_at_transfers", [B_LOC, 4], f32, kind="ExternalOutput")
    o_td = nc.dram_tensor("total_demand", [B_LOC, 1], f32, kind="ExternalOutput")
    o_ud = nc.dram_tensor("unserved_demand", [B_LOC, 1], f32, kind="ExternalOutput")
    o_ttr = nc.dram_tensor("total_transfers", [B_LOC, 1], f32, kind="ExternalOutput")
    o_oob = nc.dram_tensor("n_stops_oob", [B_LOC, 1], f32, kind="ExternalOutput")
    o_nsv = nc.dram_tensor("n_stops_visited", [B_LOC, R], f32, kind="ExternalOutput")

    # [b, j, c] views: partition p = 16*b + j, column stream c
    def view3(t):
        return t[:].rearrange("b h w -> b (h w)").rearrange(
            "b (j c) -> b j c", j=PGRP
        )

    dm3, tt3, nt3, hp3, ott3 = map(view3, (dm, tt, ntr, hp, o_tt))

    with TileContext(nc) as tc:
        with (
            tc.tile_pool(name="big", bufs=2) as big,
            tc.tile_pool(name="persist", bufs=1) as per,
            tc.tile_pool(name="psum", bufs=1, space="PSUM") as psp,
        ):
            # ---- persistent tiles ----
            acc = per.tile([128, NTERM * NCHUNK], f32)      # STT accums
            onehot_bf = per.tile([128, B_LOC], bf16)        # partition -> batch
            onehot_f = per.tile([128, B_LOC], f32)
            ohr_all = per.tile([128, 3 * B_LOC], f32)
            nc.sync.dma_start(out=onehot_bf[:], in_=doh_bf[:])
            nc.sync.dma_start(out=onehot_f[:], in_=doh_f[:])
            nc.sync.dma_start(out=ohr_all[:], in_=dohr[:])

            psum_d = psp.tile([B_LOC, MM], f32)             # total_demand partials
            psum_fold = psp.tile([B_LOC, NTERM], f32)
            psum_oob = psp.tile([B_LOC, 1], f32)

            # ---- main stream over 9 chunks ----
            for k in range(NCHUNK):
                sl = slice(k * CHUNK, (k + 1) * CHUNK)
                t_d = big.tile([128, CHUNK], f32)
                t_tt = big.tile([128, CHUNK], f32)
                t_nt = big.tile([128, CHUNK], i32)
                t_hp = big.tile([128, CHUNK], u8)
                nc.sync.dma_start(out=t_d[:], in_=dm3[:, :, sl])
                nc.sync.dma_start(out=t_tt[:], in_=tt3[:, :, sl])
                nc.sync.dma_start(out=t_nt[:], in_=nt3[:, :, sl])
                nc.sync.dma_start(out=t_hp[:], in_=hp3[:, :, sl])

                # casts to bf16 on the scalar engine
                t_db = big.tile([128, CHUNK], bf16)
                t_hpb = big.tile([128, CHUNK], bf16)
                t_ntb = big.tile([128, CHUNK], bf16)
                nc.scalar.copy(t_db[:], t_d[:])
                nc.scalar.copy(t_hpb[:], t_hp[:])
                nc.scalar.copy(t_ntb[:], t_nt[:])

                # exact fp32 trip_times = has_path * transit_times
                t_ott = big.tile([128, CHUNK], f32)
                nc.vector.tensor_tensor(
                    out=t_ott[:], in0=t_hp[:], in1=t_tt[:], op=Alu.mult,
                )
                nc.sync.dma_start(out=ott3[:, :, sl], in_=t_ott[:])
                t_ottb = big.tile([128, CHUNK], bf16)
                nc.scalar.copy(t_ottb[:], t_ott[:])

                # fused bf16 elementwise+accumulate passes
                t_dh = big.tile([128, CHUNK], bf16)
                t_junk = big.tile([128, CHUNK], bf16)

                def acc_col(term):
                    return acc[:, term * NCHUNK + k:term * NCHUNK + k + 1]

                # dh = d*hp            -> S_dh
                nc.vector.scalar_tensor_tensor(
                    out=t_dh[:], in0=t_hpb[:], scalar=1.0, in1=t_db[:],
                    op0=Alu.mult, op1=Alu.mult, accum_out=acc_col(0),
                )
                # q = d*trip_times     -> S_dtt
                nc.vector.scalar_tensor_tensor(
                    out=t_junk[:], in0=t_ottb[:], scalar=1.0, in1=t_db[:],
                    op0=Alu.mult, op1=Alu.mult, accum_out=acc_col(1),
                )
                # r = d*n_transfers    -> S_dnt
                nc.vector.scalar_tensor_tensor(
                    out=t_junk[:], in0=t_ntb[:], scalar=1.0, in1=t_db[:],
                    op0=Alu.mult, op1=Alu.mult, accum_out=acc_col(2),
                )
                # p_i = dh*(nt==i)     -> t0,t1,t2
                for i in range(3):
                    nc.vector.scalar_tensor_tensor(
                        out=t_junk[:], in0=t_ntb[:], scalar=float(i), in1=t_dh[:],
                        op0=Alu.is_equal, op1=Alu.mult, accum_out=acc_col(3 + i),
                    )

                # exact-ish total_demand on the tensor engine (bf16 source)
                for m in range(CHUNK // MM):
                    nc.tensor.matmul(
                        psum_d[:, :],
                        onehot_bf[:],
                        t_db[:, m * MM:(m + 1) * MM],
                        start=(k == 0 and m == 0),
                        stop=(k == NCHUNK - 1 and m == CHUNK // MM - 1),
                    )

            # ---- route bookkeeping (tiny) ----
            br_flat = br[:].rearrange("b r l -> (b r) l")       # [320, 50]
            tile_rows = [(0, 120), (120, 120), (240, 80)]
            visited_tiles = []
            for t, (row0, P) in enumerate(tile_rows):
                rt = per.tile([128, L], i32, tag=f"rt{t}")
                nc.sync.dma_start(out=rt[:P], in_=br_flat[row0:row0 + P])
                rtf = per.tile([128, L], f32, tag=f"rtf{t}")
                nc.scalar.copy(rtf[:P], rt[:P])

                valid = per.tile([128, L], f32, tag=f"valid{t}")
                nc.vector.tensor_scalar(
                    out=valid[:P], in0=rtf[:P], scalar1=-1.0, scalar2=None,
                    op0=Alu.is_gt,
                )
                rlen = per.tile([128, 1], f32, tag=f"rlen{t}")
                nc.vector.reduce_sum(rlen[:P], valid[:P], axis=mybir.AxisListType.X)

                # duplicate detection: cnt[:, i] = #{j<i : r_j == r_i}
                cnt = per.tile([128, L], f32, tag=f"cnt{t}")
                eqj = per.tile([128, L], f32, tag=f"eqj{t}")
                nc.vector.memset(cnt[:P, 0:1], 0.0)
                for i in range(1, L):
                    nc.vector.tensor_scalar(
                        out=eqj[:P, 0:i], in0=rtf[:P, 0:i],
                        scalar1=rtf[:P, i:i + 1], scalar2=0.0,
                        op0=Alu.is_equal, op1=Alu.add,
                        accum_out=cnt[:P, i:i + 1],
                    )
                dup01 = per.tile([128, L], f32, tag=f"dup{t}")
                nc.vector.tensor_scalar(
                    out=dup01[:P, 1:L], in0=cnt[:P, 1:L], scalar1=1.0,
                    scalar2=None, op0=Alu.min,
                )
                vds = per.tile([128, 1], f32, tag=f"vds{t}")
                nc.vector.scalar_tensor_tensor(
                    out=eqj[:P, 1:L], in0=dup01[:P, 1:L], scalar=1.0,
                    in1=valid[:P, 1:L], op0=Alu.mult, op1=Alu.mult,
                    accum_out=vds[:P],
                )
                visited = per.tile([128, 1], f32, tag=f"vis{t}")
                nc.vector.scalar_tensor_tensor(
                    out=visited[:P], in0=vds[:P], scalar=-1.0, in1=rlen[:P],
                    op0=Alu.mult, op1=Alu.add,
                )
                visited_tiles.append((visited, P))

                # delta = [len==1] + max(len-48, 0)
                dlo = per.tile([128, 1], f32, tag=f"dlo{t}")
                nc.vector.tensor_scalar(
                    out=dlo[:P], in0=rlen[:P], scalar1=1.0, scalar2=None,
                    op0=Alu.is_equal,
                )
                dhi = per.tile([128, 1], f32, tag=f"dhi{t}")
                nc.vector.tensor_scalar(
                    out=dhi[:P], in0=rlen[:P], scalar1=-float(MAX_ROUTE_LEN),
                    scalar2=0.0, op0=Alu.add, op1=Alu.max,
                )
                delta = per.tile([128, 1], f32, tag=f"dta{t}")
                nc.vector.tensor_tensor(
                    out=delta[:P], in0=dlo[:P], in1=dhi[:P], op=Alu.add,
                )
                # one-hot partition -> global batch (3 batches per 120-row tile)
                nc.tensor.matmul(
                    psum_oob[:, :], ohr_all[:P, t * B_LOC:(t + 1) * B_LOC],
                    delta[:P], start=(t == 0), stop=(t == 2),
                )

            # ---- final folds / assembly ----
            acc3 = acc[:].rearrange("p (t k) -> p t k", k=NCHUNK)
            red = per.tile([128, NTERM], f32)
            nc.vector.reduce_sum(red[:], acc3, axis=mybir.AxisListType.X)
            nc.tensor.matmul(psum_fold[:, :], onehot_f[:], red[:],
                             start=True, stop=True)

            terms = per.tile([B_LOC, NTERM], f32)
            nc.vector.tensor_copy(out=terms[:], in_=psum_fold[:, :])
            sd = per.tile([B_LOC, 1], f32)
            nc.vector.reduce_sum(sd[:], psum_d[:, :], axis=mybir.AxisListType.X)

            # unserved = S_d - S_dh
            ud = per.tile([B_LOC, 1], f32)
            nc.vector.scalar_tensor_tensor(
                out=ud[:], in0=terms[:, 0:1], scalar=-1.0, in1=sd[:],
                op0=Alu.mult, op1=Alu.add,
            )
            # trips_at_transfers = [t0, t1, t2, S_d - t0 - t1 - t2]
            tat = per.tile([B_LOC, 4], f32)
            nc.vector.tensor_copy(out=tat[:, 0:3], in_=terms[:, 3:6])
            s3 = per.tile([B_LOC, 1], f32)
            nc.vector.reduce_sum(s3[:], terms[:, 3:6], axis=mybir.AxisListType.X)
            nc.vector.scalar_tensor_tensor(
                out=tat[:, 3:4], in0=s3[:], scalar=-1.0, in1=sd[:],
                op0=Alu.mult, op1=Alu.add,
            )
            # n_stops_oob = fold(delta) + 2*(n_routes_left - has_current)
            nrl_t = per.tile([B_LOC, 1], i32)
            hcr_t = per.tile([B_LOC, 1], u8)
            nc.sync.dma_start(out=nrl_t[:], in_=nrl[:])
            nc.sync.dma_start(out=hcr_t[:], in_=hcr[:])
            nrl_f = per.tile([B_LOC, 1], f32)
            hcr_f = per.tile([B_LOC, 1], f32)
            nc.scalar.copy(nrl_f[:], nrl_t[:])
            nc.scalar.copy(hcr_f[:], hcr_t[:])
            un = per.tile([B_LOC, 1], f32)
            nc.vector.scalar_tensor_tensor(
                out=un[:], in0=hcr_f[:], scalar=-1.0, in1=nrl_f[:],
                op0=Alu.mult, op1=Alu.add,
            )
            oob_s = per.tile([B_LOC, 1], f32)
            nc.vector.tensor_copy(out=oob_s[:], in_=psum_oob[:, :])
            oob_t = per.tile([B_LOC, 1], f32)
            nc.vector.scalar_tensor_tensor(
                out=oob_t[:], in0=un[:], scalar=float(MIN_ROUTE_LEN), in1=oob_s[:],
                op0=Alu.mult, op1=Alu.add,
            )

            # ---- small output DMAs ----
            nc.sync.dma_start(out=o_tdt[:], in_=terms[:, 1:2])
            nc.sync.dma_start(out=o_ttr[:], in_=terms[:, 2:3])
            nc.sync.dma_start(out=o_td[:], in_=sd[:])
            nc.sync.dma_start(out=o_ud[:], in_=ud[:])
            nc.sync.dma_start(out=o_tat[:], in_=tat[:])
            nc.sync.dma_start(out=o_oob[:], in_=oob_t[:])
            nsv_flat = o_nsv[:].rearrange("b r -> (b r)")
            for (visited, P), (row0, _) in zip(visited_tiles, tile_rows):
                nc.sync.dma_start(out=nsv_flat[row0:row0 + P], in_=visited[:P, 0:1])

    _split_multi_waits(nc)
    return nc


def _get_nc():
    if "nc" not in _CACHE:
        _CACHE["nc"] = _build_nc()
    return _CACHE["nc"]


def _onehots():
    import ml_dtypes
    oh16 = np.zeros((128, B_LOC), np.float32)
    for p in range(128):
        oh16[p, p // PGRP] = 1.0
    ohr = np.zeros((128, 3 * B_LOC), np.float32)
    for t, P in enumerate((120, 120, 80)):
        for p in range(P):
            ohr[p, t * B_LOC + t * 3 + p // R] = 1.0
    return oh16.astype(ml_dtypes.bfloat16), oh16, ohr


def _shard_inputs(inputs):
    """Build per-core in_maps from the full input dict."""
    oh_bf, oh_f, ohr = _onehots()
    dm = np.ascontiguousarray(inputs["demand"], dtype=np.float32)
    tt = np.ascontiguousarray(inputs["transit_times"], dtype=np.float32)
    nt = np.ascontiguousarray(inputs["n_transfers"], dtype=np.int32)
    hp = np.ascontiguousarray(inputs["has_path"]).view(np.uint8)
    br = np.ascontiguousarray(inputs["batch_routes"], dtype=np.int32)
    nrl = np.ascontiguousarray(
        inputs["n_routes_left_to_plan"], dtype=np.int32).reshape(B, 1)
    hcr = np.ascontiguousarray(
        inputs["has_current_route"]).view(np.uint8).reshape(B, 1)
    in_maps = []
    for c in range(N_CORES):
        s = slice(c * B_LOC, (c + 1) * B_LOC)
        in_maps.append({
            "demand": dm[s],
            "transit_times": tt[s],
            "n_transfers": nt[s],
            "has_path": hp[s],
            "batch_routes": br[s],
            "n_routes_left_to_plan": nrl[s],
            "has_current_route": hcr[s],
            "oh16_bf16": oh_bf,
            "oh16_f32": oh_f,
            "ohr40": ohr,
        })
    return in_maps


def kernel(**inputs):
    global LAST_RESULTS
    from concourse import bass_utils

    nc = _get_nc()
    in_maps = _shard_inputs(inputs)
    trace = os.environ.get("BASS_KERNEL_TRACE", "0") == "1"
    res = bass_utils.run_bass_kernel_spmd(
        nc, in_maps, core_ids=list(range(N_CORES)), trace=trace,
    )
    LAST_RESULTS = res
    outs = res.results

    def gather(name):
        return np.concatenate([outs[c][name] for c in range(N_CORES)], axis=0)

    total_demand_time = gather("total_demand_time").reshape(B)
    trips_at_transfers = gather("trips_at_transfers")
    total_demand = gather("total_demand").reshape(B)
    unserved_demand = gather("unserved_demand").reshape(B)
    total_transfers = gather("total_transfers").reshape(B)
    trip_times = gather("trip_times")
    n_stops_oob = gather("n_stops_oob").reshape(B)
    n_stops_visited = gather("n_stops_visited")
    total_route_time = np.ascontiguousarray(
        inputs["total_route_time"], dtype=np.float32)

    return (total_demand_time, total_route_time, trips_at_transfers,
            total_demand, unserved_demand, total_transfers, trip_times,
            n_stops_oob, n_stops_visited)
